# revision 1
# baseline (speedup 1.0000x reference)
"""BertSelfAttention on 8 Trainium2 NeuronCores.

Sharding: data parallel over batch (B=2) x tensor parallel over heads
(16 heads -> 4 groups of 4). Core c handles batch c//4, heads 4*(c%4)..+4.
No collectives needed: each core produces a disjoint [256, 2048] slice of
the output (feature-major) which the host transposes/concatenates.

Per-core device program (identical on all cores, SPMD over data):
  inputs (host-prepped):
    xt    [1024, 2048]  hidden_states[b].T          (f32r)
    wq/wk/wv [1024, 256] weight column slices (wq,qb pre-scaled by 1/8)
    qb2/kb2 [128, 2]    bias chunks (per-partition layout)
    vb    [1, 256]
    maskc [128, 16]     additive mask chunks (mask[c*128+p] at [p, c])
  output:
    out   [256, 2048]   context slice, feature-major (host transposes)

  Stage A (projections, PE, f32r single-pass matmuls):
    Q.T, K.T feature-major  [128 feats(2 heads), 2048 tokens]
    V token-major [128 tokens x 16 tiles, 4*(64+ones col)]  (bf16)
  Stage B (attention per head):
    S_T[k, q] = K_h-tile.T @ Q_h   (PE, f32r, contraction over d=64)
    expS = exp(S_T + mask_k)       (ACT, mask as per-partition bias, ->bf16)
    ctxT/denom = V_aug-tile.T @ expS summed over k  (PE bf16, M=65:
                 rows 0-63 unnormalized ctx.T, row 64 softmax denom)
    normalize: DVE reciprocal of denom row + DMA partition-broadcast +
    DVE multiply; result stays feature-major [64, 2048] per head.
"""

import numpy as np

HIDDEN = 1024
HEADS = 16
HD = 64
B = 2
S = 2048
NCORES = 8
HPC = HEADS // 4  # heads per core = 4
WCOLS = HPC * HD  # 256 weight columns per core

_CACHE = {}


def _build_program():
    import concourse.bass as bass
    import concourse.bacc as bacc
    import concourse.tile as tile
    import concourse.mybir as mybir

    f32 = mybir.dt.float32
    f32r = mybir.dt.float32r
    bf16 = mybir.dt.bfloat16

    nc = bacc.Bacc("TRN2", target_bir_lowering=False, debug=False, num_devices=NCORES)

    xt_d = nc.dram_tensor("xt", [HIDDEN, S], f32r, kind="ExternalInput")
    wq_d = nc.dram_tensor("wq", [HIDDEN, WCOLS], f32r, kind="ExternalInput")
    wk_d = nc.dram_tensor("wk", [HIDDEN, WCOLS], f32r, kind="ExternalInput")
    wv_d = nc.dram_tensor("wv", [HIDDEN, WCOLS], f32r, kind="ExternalInput")
    qb_d = nc.dram_tensor("qb2", [128, 2], f32, kind="ExternalInput")
    kb_d = nc.dram_tensor("kb2", [128, 2], f32, kind="ExternalInput")
    vb_d = nc.dram_tensor("vb", [1, WCOLS], f32, kind="ExternalInput")
    mask_d = nc.dram_tensor("maskc", [128, 16], f32, kind="ExternalInput")
    out_d = nc.dram_tensor("out", [WCOLS, S], f32, kind="ExternalOutput")

    xt_r = xt_d.ap().rearrange("(c p) m -> p c m", p=128)  # [128, 8, 2048]
    wq_r = wq_d.ap().rearrange("(c p) n -> p c n", p=128)  # [128, 8, 256]
    wk_r = wk_d.ap().rearrange("(c p) n -> p c n", p=128)
    wv_r = wv_d.ap().rearrange("(c p) n -> p c n", p=128)

    with tile.TileContext(nc) as tc:
        with tc.tile_pool(name="persist", bufs=1) as persist:
            # persistent SBUF
            q_sb = persist.tile([128, 2, S], f32r)  # [feat(2 heads), mc, token]
            k_sb = persist.tile([128, 2, S], f32r)
            v_sb = persist.tile([128, 16, 4 * 65], f32r)  # [token, tile, 4*(64+one)]
            qkb = persist.tile([128, 20], f32)
            qb_sb = qkb[:, 0:2]
            kb_sb = qkb[:, 2:4]
            mask_sb = qkb[:, 4:20]
            v_blk = v_sb.rearrange("p m (l c) -> p m l c", l=4)
            vst = persist.tile([128, 32, 4], f32)
            nc.vector.memset(vst[:], 1.0)
            nc.vector.tensor_copy(v_blk[:, :, :, 0], vst[:, 0:16, :])
            ones_sb = vst.rearrange("p a b -> p (a b)")[0:1, 0:128]

            with (
                tc.tile_pool(name="wkp", bufs=1) as wkp,
            ):
              with (
                tc.tile_pool(name="proj", bufs=1) as proj,
                tc.tile_pool(name="ps_big", bufs=1, space="PSUM") as ps_big,
                tc.tile_pool(name="ps_sm", bufs=1, space="PSUM") as ps_sm,
              ):
                  xt = [proj.tile([128, S], f32r, tag=f"xt{k}", name=f"xt{k}") for k in range(8)]
                  wq_sb = proj.tile([128, 8, WCOLS], f32r)
                  wk_sb = proj.tile([128, 8, WCOLS], f32r)
                  # DMA order matters: wq first, then xt chunks, so Q-proj can
                  # start as soon as chunk 0 lands.
                  nc.sync.dma_start(out=wq_sb[:], in_=wq_r)
                  for k in range(8):
                      nc.sync.dma_start(out=xt[k][:], in_=xt_r[:, k, :])
                  nc.sync.dma_start(out=wk_sb[:], in_=wk_r)
                  nc.sync.dma_start(out=qb_sb, in_=qb_d.ap())
                  nc.sync.dma_start(out=kb_sb, in_=kb_d.ap())
                  nc.sync.dma_start(out=mask_sb, in_=mask_d.ap())

                  def proj_group(w_sb, b_sb, dst, mc, sp):
                      pq = ps_big.tile([128, 512], f32, tag="pq")
                      for k in range(8):
                          nc.tensor.matmul(
                              pq[:],
                              lhsT=w_sb[:, k, mc * 128 : mc * 128 + 128],
                              rhs=xt[k][:, sp * 512 : sp * 512 + 512],
                              start=(k == 0),
                              stop=(k == 7),
                          )
                      nc.vector.tensor_scalar_add(
                          dst[:, mc, sp * 512 : sp * 512 + 512],
                          pq[:],
                          b_sb[:, mc : mc + 1],
                      )

                  def v_group(mt, wv_sb=None, vb_sb=None):
                      pv = ps_big.tile([128, 512], f32, tag="pq", name="pv")[:, 0:256]
                      for k in range(8):
                          nc.tensor.matmul(
                              pv[:],
                              lhsT=xt[k][:, mt * 128 : mt * 128 + 128],
                              rhs=wv_sb[:, k, :],
                              start=(k == 0),
                              stop=False,
                          )
                      nc.tensor.matmul(
                          pv[:],
                          lhsT=ones_sb,
                          rhs=vb_sb[0:1, :],
                          start=False,
                          stop=True,
                      )
                      for lh in range(4):
                          nc.vector.tensor_copy(
                              v_sb[:, mt, 65 * lh + 1 : 65 * lh + 65],
                              pv[:, 64 * lh : 64 * lh + 64],
                          )

                  def s_pair(mc, sp, expP, kt, pool=None):
                      qs = sp * 512
                      ps = (pool or ps_big).tile([128, 1024], f32, tag="ps", bufs=2)
                      for half in range(2):
                          rs = 64 * half
                          nc.tensor.matmul(
                              ps[:, half * 512 : half * 512 + 512],
                              lhsT=k_sb[rs : rs + 64, mc, kt * 128 : kt * 128 + 128],
                              rhs=q_sb[rs : rs + 64, mc, qs : qs + 512],
                              start=True,
                              stop=True,
                          )
                      nc.scalar.activation(
                          expP[:, kt, :],
                          ps[:],
                          mybir.ActivationFunctionType.Exp,
                          bias=mask_sb[:, kt : kt + 1],
                      )

                  def ctx_head(mc, sp, expP, half, pool=None, pc_bufs=2):
                      lh = 2 * mc + half
                      qs = sp * 512
                      pc = (pool or ps_sm).tile([65, 512], f32, tag="pc", bufs=pc_bufs)
                      for kt in range(16):
                          nc.tensor.matmul(
                              pc[:],
                              lhsT=v_sb[:, kt, 65 * lh : 65 * lh + 65],
                              rhs=expP[:, kt, half * 512 : half * 512 + 512],
                              start=(kt == 0),
                              stop=(kt == 15),
                          )
                      ctxs = wkp.tile([65, 512], f32, tag="ctxs", bufs=2)
                      nc.vector.reciprocal(ctxs[0:1, :], pc[0:1, :])
                      bc = wkp.tile([65, 512], f32, tag="bc")
                      nc.gpsimd.partition_broadcast(bc[:], ctxs[0:1, :])
                      nc.vector.tensor_mul(ctxs[:], pc[:], bc[:])
                      nc.sync.dma_start(
                          out=out_d.ap()[64 * lh : 64 * lh + 64, qs : qs + 512],
                          in_=ctxs[1:65, :],
                      )

                  # ---- Stage A for pair 0, V for all heads ----
                  for sp in range(4):
                      proj_group(wq_sb, qb_sb, q_sb, 0, sp)
                  for sp in range(4):
                      proj_group(wk_sb, kb_sb, k_sb, 0, sp)
                  # ---- attention(pair 0) interleaved with V-proj + pair-1
                  # projections (PE filler while ACT chews the exps; Tile's
                  # slice-level deps let ctx(0,sp) start as V tiles land)
                  with (
                      tc.tile_pool(name="wvp", bufs=1) as wvp,
                      tc.tile_pool(name="att1", bufs=1) as att1,
                  ):
                      wv_sb = wvp.tile([128, 8, WCOLS], f32r)
                      vb_sb = wvp.tile([1, WCOLS], f32)
                      nc.sync.dma_start(out=wv_sb[:], in_=wv_r)
                      nc.sync.dma_start(out=vb_sb[:], in_=vb_d.ap())
                      for mt in range(16):
                          v_group(mt, wv_sb, vb_sb)
                      for sp in range(4):
                          expP = att1.tile([128, 16, 1024], f32r, tag="expP")
                          for kt in range(16):
                              s_pair(0, sp, expP, kt)
                          proj_group(wq_sb, qb_sb, q_sb, 1, sp)
                          proj_group(wk_sb, kb_sb, k_sb, 1, sp)
                          ctx_head(0, sp, expP, 0, pc_bufs=3)
                          ctx_head(0, sp, expP, 1, pc_bufs=3)

              # proj pool (xt + weights) is closed here; reuse the space for a
              # double-buffered expP so pair-1 S(sp+1) overlaps ctx(sp).
              with (
                  tc.tile_pool(name="att2", bufs=2) as att2,
                  tc.tile_pool(name="ps_big2", bufs=1, space="PSUM") as ps_big2,
                  tc.tile_pool(name="ps_sm2", bufs=1, space="PSUM") as ps_sm2,
              ):
                  expPs = {}
                  for sp in range(4):
                      expP = att2.tile([128, 16, 1024], f32r, tag="expP2")
                      expPs[sp] = expP
                      for kt in range(16):
                          s_pair(1, sp, expP, kt, pool=ps_big2)
                      if sp > 0:
                          ctx_head(1, sp - 1, expPs[sp - 1], 0, pool=ps_sm2, pc_bufs=4)
                          ctx_head(1, sp - 1, expPs[sp - 1], 1, pool=ps_sm2, pc_bufs=4)
                  ctx_head(1, 3, expPs[3], 0, pool=ps_sm2, pc_bufs=4)
                  ctx_head(1, 3, expPs[3], 1, pool=ps_sm2, pc_bufs=4)

    nc.compile()
    return nc


def _get_program():
    if "nc" not in _CACHE:
        _CACHE["nc"] = _build_program()
    return _CACHE["nc"]


def _make_in_maps(hidden_states, attention_mask, q_w, q_b, k_w, k_b, v_w, v_b):
    hs = np.asarray(hidden_states, np.float32)
    am = np.asarray(attention_mask, np.float32)
    q_w = np.asarray(q_w, np.float32)
    k_w = np.asarray(k_w, np.float32)
    v_w = np.asarray(v_w, np.float32)
    q_b = np.asarray(q_b, np.float32)
    k_b = np.asarray(k_b, np.float32)
    v_b = np.asarray(v_b, np.float32)

    scale = np.float32(1.0 / np.sqrt(HD))

    in_maps = []
    for c in range(NCORES):
        b = c // 4
        hg = c % 4
        cols = slice(WCOLS * hg, WCOLS * hg + WCOLS)
        mask = am[b, 0, 0, :]  # [S]
        in_maps.append(
            {
                "xt": np.ascontiguousarray(hs[b].T),
                "wq": np.ascontiguousarray(q_w[:, cols] * scale),
                "wk": np.ascontiguousarray(k_w[:, cols]),
                "wv": np.ascontiguousarray(v_w[:, cols]),
                "qb2": np.ascontiguousarray((q_b[cols] * scale).reshape(2, 128).T),
                "kb2": np.ascontiguousarray(k_b[cols].reshape(2, 128).T),
                "vb": np.ascontiguousarray(v_b[cols].reshape(1, WCOLS)),
                "maskc": np.ascontiguousarray(mask.reshape(16, 128).T),
            }
        )
    return in_maps


def kernel(hidden_states, attention_mask, q_w, q_b, k_w, k_b, v_w, v_b):
    from concourse import bass_utils

    nc = _get_program()
    in_maps = _make_in_maps(
        hidden_states, attention_mask, q_w, q_b, k_w, k_b, v_w, v_b
    )
    res = bass_utils.run_bass_kernel_spmd(nc, in_maps, core_ids=list(range(NCORES)))

    full = np.empty((B, S, HIDDEN), np.float32)
    for c in range(NCORES):
        b = c // 4
        hg = c % 4
        full[b, :, WCOLS * hg : WCOLS * hg + WCOLS] = res.results[c]["out"].T
    return full



# revision 2
# speedup vs baseline: 1.5945x; 1.5945x over previous
"""BertSelfAttention on 8 Trainium2 NeuronCores.

Sharding: data parallel over batch (B=2) x tensor parallel over heads
(16 heads -> 4 groups of 4). Core c handles batch c//4, heads 4*(c%4)..+4.
No collectives needed: each core produces a disjoint [256, 2048] slice of
the output (feature-major) which the host transposes/concatenates.

Per-core device program (identical on all cores, SPMD over data):
  inputs (host-prepped):
    xt    [1024, 2048]  hidden_states[b].T          (bf16)
    wq/wk/wv [1024, 256] weight column slices, bf16 (wq,qb pre-scaled by 1/8)
    qb2/kb2 [128, 2]    bias chunks (per-partition layout, f32)
    vb    [1, 256]      bf16
    maskc [128, 16]     additive mask chunks (mask[c*128+p] at [p, c], f32)
  output:
    out   [256, 2048]   context slice, feature-major (host transposes)

  Stage A (projections, PE, bf16 single-pass matmuls, f32 PSUM accumulate):
    Q.T, K.T feature-major  [128 feats(2 heads), 2048 tokens] bf16
    V token-major [128 tokens x 16 tiles, 4*(64+ones col)]  bf16
  Stage B (attention per head):
    S_T[k, q] = K_h-tile.T @ Q_h   (PE bf16, contraction over d=64)
    expS = exp(S_T + mask_k)       (ACT, mask as per-partition bias, ->bf16)
    ctxT/denom = V_aug-tile.T @ expS summed over k  (PE bf16, M=65:
                 rows 0-63 unnormalized ctx.T, row 64 softmax denom)
    normalize: DVE reciprocal_approx_fast of denom row + DMA partition-
    broadcast + DVE multiply; result stays feature-major [64, 2048]/head.
"""

import numpy as np

HIDDEN = 1024
HEADS = 16
HD = 64
B = 2
S = 2048
NCORES = 8
HPC = HEADS // 4  # heads per core = 4
WCOLS = HPC * HD  # 256 weight columns per core

_CACHE = {}


def _build_program():
    import concourse.bass as bass
    import concourse.bacc as bacc
    import concourse.tile as tile
    import concourse.mybir as mybir

    f32 = mybir.dt.float32
    bf16 = mybir.dt.bfloat16

    nc = bacc.Bacc("TRN2", target_bir_lowering=False, debug=False, num_devices=NCORES)

    xt_d = nc.dram_tensor("xt", [HIDDEN, S], bf16, kind="ExternalInput")
    wq_d = nc.dram_tensor("wq", [HIDDEN, WCOLS], bf16, kind="ExternalInput")
    wk_d = nc.dram_tensor("wk", [HIDDEN, WCOLS], bf16, kind="ExternalInput")
    wv_d = nc.dram_tensor("wv", [HIDDEN, WCOLS], bf16, kind="ExternalInput")
    qb_d = nc.dram_tensor("qb2", [128, 2], f32, kind="ExternalInput")
    kb_d = nc.dram_tensor("kb2", [128, 2], f32, kind="ExternalInput")
    vb_d = nc.dram_tensor("vb", [1, WCOLS], bf16, kind="ExternalInput")
    mask_d = nc.dram_tensor("maskc", [128, 16], f32, kind="ExternalInput")
    out_d = nc.dram_tensor("out", [WCOLS, S], f32, kind="ExternalOutput")

    xt_r = xt_d.ap().rearrange("(c p) m -> p c m", p=128)  # [128, 8, 2048]
    wq_r = wq_d.ap().rearrange("(c p) n -> p c n", p=128)  # [128, 8, 256]
    wk_r = wk_d.ap().rearrange("(c p) n -> p c n", p=128)
    wv_r = wv_d.ap().rearrange("(c p) n -> p c n", p=128)

    with tile.TileContext(nc) as tc:
        with tc.tile_pool(name="persist", bufs=1) as persist:
            # persistent SBUF
            q_sb = persist.tile([128, 2, S], bf16)  # [feat(2 heads), mc, token]
            k_sb = persist.tile([128, 2, S], bf16)
            v_sb = persist.tile([128, 16, 4 * 65], bf16)  # [token, tile, 4*(64+one)]
            qkb = persist.tile([128, 20], f32)
            qb_sb = qkb[:, 0:2]
            kb_sb = qkb[:, 2:4]
            mask_sb = qkb[:, 4:20]
            v_blk = v_sb.rearrange("p m (l c) -> p m l c", l=4)
            nc.vector.memset(v_blk[:, :, :, 0], 1.0)  # ones cols for denom rows
            ones_t = persist.tile([1, 128], bf16)
            nc.vector.memset(ones_t[:], 1.0)
            ones_sb = ones_t[0:1, 0:128]

            with (
                tc.tile_pool(name="wkp", bufs=1) as wkp,
            ):
              with (
                tc.tile_pool(name="proj", bufs=1) as proj,
                tc.tile_pool(name="ps_big", bufs=1, space="PSUM") as ps_big,
                tc.tile_pool(name="ps_sm", bufs=1, space="PSUM") as ps_sm,
              ):
                  xt = [proj.tile([128, S], bf16, tag=f"xt{k}", name=f"xt{k}") for k in range(8)]
                  wq_sb = proj.tile([128, 8, WCOLS], bf16)
                  wk_sb = proj.tile([128, 8, WCOLS], bf16)
                  # DMA order matters: wq first, then xt chunks, so Q-proj can
                  # start as soon as chunk 0 lands.
                  nc.sync.dma_start(out=wq_sb[:], in_=wq_r)
                  for k in range(8):
                      nc.sync.dma_start(out=xt[k][:], in_=xt_r[:, k, :])
                  nc.sync.dma_start(out=wk_sb[:], in_=wk_r)
                  nc.sync.dma_start(out=qb_sb, in_=qb_d.ap())
                  nc.sync.dma_start(out=kb_sb, in_=kb_d.ap())
                  nc.sync.dma_start(out=mask_sb, in_=mask_d.ap())

                  def proj_group(w_sb, b_sb, dst, mc, sp):
                      pq = ps_big.tile([128, 512], f32, tag="pq")
                      for k in range(8):
                          nc.tensor.matmul(
                              pq[:],
                              lhsT=w_sb[:, k, mc * 128 : mc * 128 + 128],
                              rhs=xt[k][:, sp * 512 : sp * 512 + 512],
                              start=(k == 0),
                              stop=(k == 7),
                          )
                      nc.vector.tensor_scalar_add(
                          dst[:, mc, sp * 512 : sp * 512 + 512],
                          pq[:],
                          b_sb[:, mc : mc + 1],
                      )

                  def v_group(mt, wv_sb=None, vb_sb=None):
                      pv = ps_big.tile([128, 512], f32, tag="pq", name="pv")[:, 0:256]
                      for k in range(8):
                          nc.tensor.matmul(
                              pv[:],
                              lhsT=xt[k][:, mt * 128 : mt * 128 + 128],
                              rhs=wv_sb[:, k, :],
                              start=(k == 0),
                              stop=False,
                          )
                      nc.tensor.matmul(
                          pv[:],
                          lhsT=ones_sb,
                          rhs=vb_sb[0:1, :],
                          start=False,
                          stop=True,
                      )
                      nc.vector.tensor_copy(
                          v_blk[:, mt, :, 1:65],
                          pv.rearrange("p (l c) -> p l c", l=4),
                      )

                  def s_pair(mc, sp, expP, kt, pool=None):
                      qs = sp * 512
                      ps = (pool or ps_big).tile([128, 1024], f32, tag="ps", bufs=2)
                      for half in range(2):
                          rs = 64 * half
                          nc.tensor.matmul(
                              ps[:, half * 512 : half * 512 + 512],
                              lhsT=k_sb[rs : rs + 64, mc, kt * 128 : kt * 128 + 128],
                              rhs=q_sb[rs : rs + 64, mc, qs : qs + 512],
                              start=True,
                              stop=True,
                          )
                      nc.scalar.activation(
                          expP[:, kt, :],
                          ps[:],
                          mybir.ActivationFunctionType.Exp,
                          bias=mask_sb[:, kt : kt + 1],
                      )

                  def ctx_head(mc, sp, expP, half, pool=None, pc_bufs=2):
                      lh = 2 * mc + half
                      qs = sp * 512
                      pc = (pool or ps_sm).tile([65, 512], f32, tag="pc", bufs=pc_bufs)
                      for kt in range(16):
                          nc.tensor.matmul(
                              pc[:],
                              lhsT=v_sb[:, kt, 65 * lh : 65 * lh + 65],
                              rhs=expP[:, kt, half * 512 : half * 512 + 512],
                              start=(kt == 0),
                              stop=(kt == 15),
                          )
                      ctxs = wkp.tile([65, 512], f32, tag="ctxs", bufs=2)
                      nc.vector.reciprocal_approx_fast(ctxs[0:1, :], pc[0:1, :])
                      bc = wkp.tile([65, 512], f32, tag="bc")
                      nc.gpsimd.partition_broadcast(bc[:], ctxs[0:1, :])
                      nc.vector.tensor_mul(ctxs[:], pc[:], bc[:])
                      nc.sync.dma_start(
                          out=out_d.ap()[64 * lh : 64 * lh + 64, qs : qs + 512],
                          in_=ctxs[1:65, :],
                      )

                  # ---- Stage A for pair 0, V for all heads ----
                  for sp in range(4):
                      proj_group(wq_sb, qb_sb, q_sb, 0, sp)
                  for sp in range(4):
                      proj_group(wk_sb, kb_sb, k_sb, 0, sp)
                  # ---- attention(pair 0) interleaved with V-proj + pair-1
                  # projections (PE filler while ACT chews the exps; Tile's
                  # slice-level deps let ctx(0,sp) start as V tiles land)
                  with (
                      tc.tile_pool(name="wvp", bufs=1) as wvp,
                      tc.tile_pool(name="att1", bufs=1) as att1,
                  ):
                      wv_sb = wvp.tile([128, 8, WCOLS], bf16)
                      vb_sb = wvp.tile([1, WCOLS], bf16)
                      nc.sync.dma_start(out=wv_sb[:], in_=wv_r)
                      nc.sync.dma_start(out=vb_sb[:], in_=vb_d.ap())
                      for mt in range(16):
                          v_group(mt, wv_sb, vb_sb)
                      for sp in range(4):
                          expP = att1.tile([128, 16, 1024], bf16, tag="expP")
                          for kt in range(16):
                              s_pair(0, sp, expP, kt)
                          proj_group(wq_sb, qb_sb, q_sb, 1, sp)
                          proj_group(wk_sb, kb_sb, k_sb, 1, sp)
                          ctx_head(0, sp, expP, 0, pc_bufs=3)
                          ctx_head(0, sp, expP, 1, pc_bufs=3)

              # proj pool (xt + weights) is closed here; reuse the space for a
              # double-buffered expP so pair-1 S(sp+1) overlaps ctx(sp).
              with (
                  tc.tile_pool(name="att2", bufs=2) as att2,
                  tc.tile_pool(name="ps_big2", bufs=1, space="PSUM") as ps_big2,
                  tc.tile_pool(name="ps_sm2", bufs=1, space="PSUM") as ps_sm2,
              ):
                  expPs = {}
                  for sp in range(4):
                      expP = att2.tile([128, 16, 1024], bf16, tag="expP2")
                      expPs[sp] = expP
                      for kt in range(16):
                          s_pair(1, sp, expP, kt, pool=ps_big2)
                      if sp > 0:
                          ctx_head(1, sp - 1, expPs[sp - 1], 0, pool=ps_sm2, pc_bufs=4)
                          ctx_head(1, sp - 1, expPs[sp - 1], 1, pool=ps_sm2, pc_bufs=4)
                  ctx_head(1, 3, expPs[3], 0, pool=ps_sm2, pc_bufs=4)
                  ctx_head(1, 3, expPs[3], 1, pool=ps_sm2, pc_bufs=4)

    nc.compile()
    return nc


def _get_program():
    if "nc" not in _CACHE:
        _CACHE["nc"] = _build_program()
    return _CACHE["nc"]


def _to_bf16(x):
    import ml_dtypes

    return np.asarray(x, np.float32).astype(ml_dtypes.bfloat16)


def _make_in_maps(hidden_states, attention_mask, q_w, q_b, k_w, k_b, v_w, v_b):
    hs = np.asarray(hidden_states, np.float32)
    am = np.asarray(attention_mask, np.float32)
    q_w = np.asarray(q_w, np.float32)
    k_w = np.asarray(k_w, np.float32)
    v_w = np.asarray(v_w, np.float32)
    q_b = np.asarray(q_b, np.float32)
    k_b = np.asarray(k_b, np.float32)
    v_b = np.asarray(v_b, np.float32)

    scale = np.float32(1.0 / np.sqrt(HD))

    in_maps = []
    for c in range(NCORES):
        b = c // 4
        hg = c % 4
        cols = slice(WCOLS * hg, WCOLS * hg + WCOLS)
        mask = am[b, 0, 0, :]  # [S]
        in_maps.append(
            {
                "xt": np.ascontiguousarray(_to_bf16(hs[b].T)),
                "wq": np.ascontiguousarray(_to_bf16(q_w[:, cols] * scale)),
                "wk": np.ascontiguousarray(_to_bf16(k_w[:, cols])),
                "wv": np.ascontiguousarray(_to_bf16(v_w[:, cols])),
                "qb2": np.ascontiguousarray((q_b[cols] * scale).reshape(2, 128).T),
                "kb2": np.ascontiguousarray(k_b[cols].reshape(2, 128).T),
                "vb": np.ascontiguousarray(_to_bf16(v_b[cols].reshape(1, WCOLS))),
                "maskc": np.ascontiguousarray(mask.reshape(16, 128).T),
            }
        )
    return in_maps


def kernel(hidden_states, attention_mask, q_w, q_b, k_w, k_b, v_w, v_b):
    from concourse import bass_utils

    nc = _get_program()
    in_maps = _make_in_maps(
        hidden_states, attention_mask, q_w, q_b, k_w, k_b, v_w, v_b
    )
    _CACHE["in_maps"] = in_maps
    res = bass_utils.run_bass_kernel_spmd(nc, in_maps, core_ids=list(range(NCORES)))

    full = np.empty((B, S, HIDDEN), np.float32)
    for c in range(NCORES):
        b = c // 4
        hg = c % 4
        full[b, :, WCOLS * hg : WCOLS * hg + WCOLS] = res.results[c]["out"].T
    return full
